# revision 1
# baseline (speedup 1.0000x reference)
"""Trainium2 Bass kernel for grouped-expert 3-layer MLP (MoE, known covariance).

Computes, for x[B, E, DIN] and per-expert weights:
    h1 = relu(x[:,e] @ W1[e] + b1[e])      # [B, H]
    h2 = relu(h1 @ W2[e] + b2[e])          # [B, H]
    o  = h2 @ W3[e] + b3[e]                # [B, DOUT]
    out = sum_e o                          # [B, DOUT]

Sharding: data-parallel over batch across 8 NeuronCores (B=8192 -> 1024/core).
Weights are replicated to every core; no collectives needed.

Per-core layout strategy (feature-major activations):
  The PE contracts along the partition dim, and x stores DIN minor, so x is
  transposed on-chip (PE transpose via identity) into xT [DIN, batch] tiles.
  All three layers then run with the weight panel natural-layout as the
  stationary operand and activations streaming feature-major:
     h1T[hb]  = relu(W1[:,hb].T @ xT + b1)        4 blocks of [128, NB]
     h2T[gb]  = relu(sum_hb W2[hb,gb].T @ h1T[hb] + b2)
     oT      += sum_gb W3[gb].T @ h2T[gb]         accumulated in PSUM over all
                                                  experts (one bank per batch tile)
  Finally oT gets the summed b3 bias, is PE-transposed back to batch-major and
  DMA'd out contiguously.

Matmuls run as float32r (fp32 bits, fast PE mode: 1 cycle/row at N>=256).
"""

import os
from contextlib import ExitStack

import bass_rust
import numpy as np

import concourse.bass as bass
import concourse.tile as tile
from concourse import bacc, mybir
from concourse.bass_utils import run_bass_kernel_spmd
from concourse.masks import make_identity

E, DIN, H, DOUT = 16, 128, 512, 64
B_FULL = 8192
N_CORES = 8
HB = H // 128  # 4 h-blocks
F32 = mybir.dt.float32
FR = mybir.dt.float32r


def build_nc(bloc=B_FULL // N_CORES, nb=512, l1_dt=FR, l23_dt=FR):
    """Build the per-core Bass program. bloc = local batch, nb = batch tile.

    l1_dt: matmul dtype for the x-transposes + layer 1 (FR or bfloat16).
    l23_dt: matmul dtype for layers 2 and 3.
    float32r (TF32 on the PE, 1 cycle/row at N>=256 vs 4 for fp32) requires
    every producer of matmul-consumed data to write rounded output: weights
    are cast on the fly by SWDGE DMA (gpsimd descriptors support dtype
    conversion), activations are rounded by the ACT/DVE evacuation op.
    """
    nbt = bloc // nb
    nt = nb // 128  # 128-row chunks per batch tile
    assert bloc % nb == 0 and nb % 128 == 0

    nc = bacc.Bacc("TRN2", target_bir_lowering=False, debug=False)

    x = nc.dram_tensor("x", [bloc, E, DIN], F32, kind="ExternalInput")
    W1 = nc.dram_tensor("W1", [E, DIN, H], F32, kind="ExternalInput")
    b1 = nc.dram_tensor("b1", [E, H], F32, kind="ExternalInput")
    W2 = nc.dram_tensor("W2", [E, H, H], F32, kind="ExternalInput")
    b2 = nc.dram_tensor("b2", [E, H], F32, kind="ExternalInput")
    W3 = nc.dram_tensor("W3", [E, H, DOUT], F32, kind="ExternalInput")
    b3 = nc.dram_tensor("b3", [E, DOUT], F32, kind="ExternalInput")
    out = nc.dram_tensor("out", [bloc, DOUT], F32, kind="ExternalOutput")

    RELU = mybir.ActivationFunctionType.Relu
    ADD = mybir.AluOpType.add
    MAX = mybir.AluOpType.max

    with TileCtx(nc) as tc, ExitStack() as ctx:
        consts = ctx.enter_context(tc.tile_pool(name="consts", bufs=1))
        w1p = ctx.enter_context(tc.tile_pool(name="w1p", bufs=2))
        w2p = ctx.enter_context(tc.tile_pool(name="w2p", bufs=2))
        w3p = ctx.enter_context(tc.tile_pool(name="w3p", bufs=2))
        xp = ctx.enter_context(tc.tile_pool(name="xp", bufs=4))
        xtp = ctx.enter_context(tc.tile_pool(name="xtp", bufs=3))
        h1p = ctx.enter_context(tc.tile_pool(name="h1p", bufs=2))
        h2p = ctx.enter_context(tc.tile_pool(name="h2p", bufs=2))
        obp = ctx.enter_context(tc.tile_pool(name="obp", bufs=2))
        pxt = ctx.enter_context(tc.tile_pool(name="pxt", bufs=2, space="PSUM"))
        p1p = ctx.enter_context(tc.tile_pool(name="p1p", bufs=2, space="PSUM"))
        p2p = ctx.enter_context(tc.tile_pool(name="p2p", bufs=2, space="PSUM"))
        pop = ctx.enter_context(tc.tile_pool(name="pop", bufs=nbt, space="PSUM"))

        ident = consts.tile([128, 128], F32)
        make_identity(nc, ident)
        identr = consts.tile([128, 128], l1_dt)
        nc.scalar.copy(identr, ident)

        # biases: load in natural layout (few large descriptors — a direct
        # rearranged DMA would be 8192 4-byte descriptors and stall the Sync
        # queue ~10us), then PE-transpose so the per-feature bias lands on
        # partitions: b1s[p, hb*E + e] = b1[e, hb*128 + p]
        b1n = consts.tile([E, H], F32)
        nc.sync.dma_start(out=b1n, in_=b1[:, :])
        b2n = consts.tile([E, H], F32)
        nc.sync.dma_start(out=b2n, in_=b2[:, :])
        b3n = consts.tile([E, DOUT], F32)
        nc.sync.dma_start(out=b3n, in_=b3[:, :])
        b1s = consts.tile([128, HB * E], F32)
        b2s = consts.tile([128, HB * E], F32)
        for bn, bs in ((b1n, b1s), (b2n, b2s)):
            pb = pxt.tile([128, HB * E], F32, tag="pxt", name="pb")
            for hb in range(HB):
                nc.tensor.transpose(
                    pb[:, hb * E : (hb + 1) * E],
                    bn[:, hb * 128 : (hb + 1) * 128],
                    ident[:E, :E],
                )
            nc.vector.tensor_copy(bs, pb)
        pb3 = pxt.tile([DOUT, E], F32, tag="pxt", name="pb3")
        nc.tensor.transpose(pb3, b3n, ident[:E, :E])
        b3s = consts.tile([DOUT, E], F32)
        nc.vector.tensor_copy(b3s, pb3)
        b3sum = consts.tile([DOUT, 1], F32)
        nc.vector.reduce_sum(b3sum, b3s, axis=bass_rust.AxisListType.X)

        # PSUM accumulators for the expert-summed output, one per batch tile,
        # alive across the whole expert loop.
        po = [pop.tile([DOUT, nb], F32, tag="po", name=f"po{i}") for i in range(nbt)]

        for e in range(E):
            wdma = nc.gpsimd
            w1t = w1p.tile([DIN, H], l1_dt, tag="w1")
            wdma.dma_start(out=w1t, in_=W1[e])
            w2t = w2p.tile([128, HB, H], l23_dt, tag="w2")
            for whb in range(HB):
                wdma.dma_start(
                    out=w2t[:, whb, :], in_=W2[e, whb * 128 : (whb + 1) * 128, :]
                )
            w3t = w3p.tile([128, HB, DOUT], l23_dt, tag="w3")
            wdma.dma_start(out=w3t, in_=W3[e].rearrange("(hb p) o -> p hb o", p=128))

            for bt in range(nbt):
                b0 = bt * nb
                # natural-layout x chunks: xin[p, t, d] = x[b0 + t*128 + p, e, d]
                xin = xp.tile([128, nt, DIN], l1_dt, tag="xin")
                nc.gpsimd.dma_start(
                    out=xin, in_=x[b0 : b0 + nb, e, :].rearrange("(t p) d -> p t d", p=128)
                )
                # transpose to feature-major xT[d, j] = x[b0 + j, e, d]
                pxt_t = pxt.tile([DIN, nb], l1_dt, tag="pxt")
                for t in range(nt):
                    nc.tensor.transpose(
                        pxt_t[:, t * 128 : (t + 1) * 128], xin[:, t, :], identr
                    )
                xt = xtp.tile([DIN, nb], l1_dt, tag="xt")
                if (e + bt) % 2 == 0:
                    nc.scalar.copy(xt, pxt_t)
                else:
                    nc.vector.tensor_copy(xt, pxt_t)

                # ---- layer 1 ----
                h1 = []
                for hb in range(HB):
                    ps = p1p.tile([128, nb], F32, tag="p1")
                    nc.tensor.matmul(
                        ps, w1t[:, hb * 128 : (hb + 1) * 128], xt, start=True, stop=True
                    )
                    ht = h1p.tile([128, nb], l23_dt, tag=f"h1_{hb}")
                    bias = b1s[:, hb * E + e : hb * E + e + 1]
                    if hb % 2 == 0:
                        nc.scalar.activation(ht, ps, RELU, bias=bias)
                    else:
                        nc.vector.tensor_scalar(ht, ps, bias, 0.0, ADD, MAX)
                    h1.append(ht)

                # ---- layer 2 ----
                h2 = []
                for gb in range(HB):
                    ps = p2p.tile([128, nb], F32, tag="p2")
                    for hb in range(HB):
                        nc.tensor.matmul(
                            ps,
                            w2t[:, hb, gb * 128 : (gb + 1) * 128],
                            h1[hb],
                            start=(hb == 0),
                            stop=(hb == HB - 1),
                        )
                    ht = h2p.tile([128, nb], l23_dt, tag=f"h2_{gb}")
                    bias = b2s[:, gb * E + e : gb * E + e + 1]
                    if gb % 2 == 1:
                        nc.scalar.activation(ht, ps, RELU, bias=bias)
                    else:
                        nc.vector.tensor_scalar(ht, ps, bias, 0.0, ADD, MAX)
                    h2.append(ht)

                # ---- layer 3: accumulate over gb and experts in PSUM ----
                for gb in range(HB):
                    nc.tensor.matmul(
                        po[bt],
                        w3t[:, gb, :],
                        h2[gb],
                        start=(e == 0 and gb == 0),
                        stop=(e == E - 1 and gb == HB - 1),
                    )

        # ---- epilogue: bias, transpose back to batch-major, store ----
        for bt in range(nbt):
            b0 = bt * nb
            ob = obp.tile([DOUT, nb], F32, tag="ob")
            nc.vector.tensor_scalar_add(ob, po[bt], b3sum)
            pot = pxt.tile([128, nt * DOUT], F32, tag="pxt")
            for t in range(nt):
                nc.tensor.transpose(
                    pot[:, t * DOUT : (t + 1) * DOUT],
                    ob[:, t * 128 : (t + 1) * 128],
                    ident[:DOUT, :DOUT],
                )
            obt = obp.tile([128, nt * DOUT], F32, tag="obt")
            nc.vector.tensor_copy(obt, pot)
            nc.sync.dma_start(
                out=out[b0 : b0 + nb, :].rearrange("(t p) o -> p t o", p=128),
                in_=obt.rearrange("p (t o) -> p t o", o=DOUT),
            )

    nc.compile()
    return nc


def TileCtx(nc):
    return tile.TileContext(nc)


_DT_MAP = {"f32r": FR, "bf16": mybir.dt.bfloat16, "f32": F32}
_NC_CACHE = {}


def _get_nc():
    l1 = os.environ.get("KERNEL_L1_DT", "f32r")
    l23 = os.environ.get("KERNEL_L23_DT", "f32r")
    key = (l1, l23)
    if key not in _NC_CACHE:
        _NC_CACHE[key] = build_nc(l1_dt=_DT_MAP[l1], l23_dt=_DT_MAP[l23])
    return _NC_CACHE[key]


def kernel(x, W1, b1, W2, b2, W3, b3):
    x = np.ascontiguousarray(np.asarray(x, dtype=np.float32))
    ws = {
        "W1": np.ascontiguousarray(np.asarray(W1, dtype=np.float32)),
        "b1": np.ascontiguousarray(np.asarray(b1, dtype=np.float32)),
        "W2": np.ascontiguousarray(np.asarray(W2, dtype=np.float32)),
        "b2": np.ascontiguousarray(np.asarray(b2, dtype=np.float32)),
        "W3": np.ascontiguousarray(np.asarray(W3, dtype=np.float32)),
        "b3": np.ascontiguousarray(np.asarray(b3, dtype=np.float32)),
    }
    nc = _get_nc()
    shards = np.split(x, N_CORES, axis=0)
    in_maps = [{"x": np.ascontiguousarray(s), **ws} for s in shards]
    trace = bool(int(os.environ.get("KERNEL_TRACE", "0")))
    kwargs = {}
    if trace and os.environ.get("KERNEL_TRACE_DIR"):
        kwargs["tmpdir"] = os.environ["KERNEL_TRACE_DIR"]
    res = run_bass_kernel_spmd(nc, in_maps, list(range(N_CORES)), trace=trace, **kwargs)
    if trace:
        kernel.last_results = res
    return np.concatenate([res.results[c]["out"] for c in range(N_CORES)], axis=0)

